# revision 7
# baseline (speedup 1.0000x reference)
"""Trainium2 Bass kernel for windowed multi-head attention with dynamic
position bias (nn_Attention_75058848465066).

Contract: kernel(**inputs) takes the FULL unsharded inputs (as produced by
reference.setup_inputs()) and returns the FULL output [16, 64, 64, 512] f32.

Strategy (hardcoded for x [16, 64, 64, 512], 8 cores):
 - Data-parallel over batch: 2 batches -> 128 windows -> 8192 tokens per core.
 - Host: fold layernorm gamma + qk scale into the projection weights, compute
   the tiny DPB MLP -> 64x64 bias table, window-partition x.
 - Device (per core): LN -> PE-transpose -> qkv projections (float32r)
   -> per-window attention with packed (tile_position) small matmuls, softmax
   via exp + ones-matmul column sums (denominator lands pre-broadcast in PSUM)
   -> output projection (float32r) -> windows written back token-major.
"""

import sys

for _p in ("/opt/trn_rl_repo",):
    if _p not in sys.path:
        sys.path.append(_p)

from contextlib import ExitStack

import numpy as np

import concourse.bass as bass
import concourse.tile as tile
from concourse import bacc, mybir
from concourse.bass_utils import run_bass_kernel_spmd
from concourse.masks import make_identity

F32 = mybir.dt.float32
F32R = mybir.dt.float32r

DIM = 512
DH = 32
WSZ = 8
HEADS = 16
D4 = DIM // 4
EPS = 1e-5

B, H, W = 16, 64, 64
NCORES = 8
BPC = B // NCORES          # batches per core
NTOK = BPC * H * W         # 8192 tokens per core
WW = WSZ * WSZ             # 64 tokens per window
NST = NTOK // 512          # 16 super-tiles of 512 tokens


# ---------------------------------------------------------------- host math
def _ln_np(x, g, b=None, eps=EPS):
    x = x.astype(np.float32)
    m = x.mean(-1, keepdims=True)
    v = x.var(-1, keepdims=True)
    y = (x - m) * (1.0 / np.sqrt(v + eps)) * g
    return y if b is None else y + b


def _dpb_bias_table(dpb_w0, dpb_b0, dpb_ln0, dpb_w1, dpb_b1, dpb_ln1,
                    dpb_w2, dpb_b2, dpb_ln2, dpb_w3, dpb_b3):
    """Returns B[i, j] = biases[rel_pos_indices[i, j]]  (shape [64, 64])."""
    w = WSZ
    pos = np.arange(w)
    gi, gj = np.meshgrid(pos, pos, indexing="ij")
    grid = np.stack([gi, gj], -1).reshape(-1, 2)
    rel = grid[:, None] - grid[None, :] + (w - 1)
    rel_idx = rel[..., 0] * (2 * w - 1) + rel[..., 1]          # [64, 64]

    p = np.arange(-w, w + 1)
    pi, pj = np.meshgrid(p, p, indexing="ij")
    rp = np.stack([pi, pj], -1).reshape(-1, 2).astype(np.float32)
    h = np.maximum(_ln_np(rp @ dpb_w0 + dpb_b0, dpb_ln0), 0.0)
    h = np.maximum(_ln_np(h @ dpb_w1 + dpb_b1, dpb_ln1), 0.0)
    h = np.maximum(_ln_np(h @ dpb_w2 + dpb_b2, dpb_ln2), 0.0)
    biases = (h @ dpb_w3 + dpb_b3)[:, 0]                       # [(2w-1+2)^2]
    return biases[rel_idx].astype(np.float32)                  # [64, 64]


def _window_partition(x):
    """[b, 64, 64, D] -> [b*64 windows * 64 tokens, D]"""
    b = x.shape[0]
    xw = x.reshape(b, H // WSZ, WSZ, W // WSZ, WSZ, DIM)
    xw = xw.transpose(0, 1, 3, 2, 4, 5)
    return np.ascontiguousarray(xw.reshape(-1, DIM))


def _window_unpartition(xw, b):
    """[b*4096, D] -> [b, 64, 64, D]"""
    x = xw.reshape(b, H // WSZ, W // WSZ, WSZ, WSZ, DIM)
    x = x.transpose(0, 1, 3, 2, 4, 5)
    return np.ascontiguousarray(x.reshape(b, H, W, DIM))


# ------------------------------------------------------------- device build
def _build_program(use_qk_bias, use_v_bias, use_o_bias, ntok=NTOK):
    nc = bacc.Bacc("TRN2", target_bir_lowering=False, debug=False,
                   num_devices=NCORES)

    xw = nc.dram_tensor("xw", [ntok, DIM], F32, kind="ExternalInput").ap()
    wq = nc.dram_tensor("wq", [DIM, DIM], F32R, kind="ExternalInput").ap()
    wk = nc.dram_tensor("wk", [DIM, DIM], F32R, kind="ExternalInput").ap()
    wv = nc.dram_tensor("wv", [DIM, DIM], F32R, kind="ExternalInput").ap()
    wo = nc.dram_tensor("wo", [DIM, DIM], F32R, kind="ExternalInput").ap()
    expb = nc.dram_tensor("expb", [128, 256], F32, kind="ExternalInput").ap()
    out = nc.dram_tensor("out", [ntok, DIM], F32, kind="ExternalOutput").ap()
    if use_qk_bias:
        bqk = nc.dram_tensor("bqk", [128, 8], F32, kind="ExternalInput").ap()
    if use_v_bias:
        bv = nc.dram_tensor("bv", [DIM], F32, kind="ExternalInput").ap()
    if use_o_bias:
        bo = nc.dram_tensor("bo", [DIM], F32, kind="ExternalInput").ap()

    with tile.TileContext(nc) as tc, ExitStack() as ctx:
        consts = ctx.enter_context(tc.tile_pool(name="consts", bufs=1))
        wpool = ctx.enter_context(tc.tile_pool(name="wpool", bufs=1))
        xpool = ctx.enter_context(tc.tile_pool(name="xpool", bufs=3))
        ypool = ctx.enter_context(tc.tile_pool(name="ypool", bufs=2))
        small = ctx.enter_context(tc.tile_pool(name="small", bufs=4))
        ytpool = ctx.enter_context(tc.tile_pool(name="ytpool", bufs=2))
        qkpool = ctx.enter_context(tc.tile_pool(name="qkpool", bufs=2))
        vpool = ctx.enter_context(tc.tile_pool(name="vpool", bufs=2))
        ppool = ctx.enter_context(tc.tile_pool(name="ppool", bufs=4))
        rspool = ctx.enter_context(tc.tile_pool(name="rspool", bufs=4))
        onpool = ctx.enter_context(tc.tile_pool(name="onpool", bufs=2))
        fpool = ctx.enter_context(tc.tile_pool(name="fpool", bufs=2))
        ps_big = ctx.enter_context(tc.tile_pool(name="ps_big", bufs=2, space="PSUM"))
        ps_sim = ctx.enter_context(tc.tile_pool(name="ps_sim", bufs=4, space="PSUM"))
        ps_avs = ctx.enter_context(tc.tile_pool(name="ps_avs", bufs=2, space="PSUM"))

        ident = consts.tile([128, 128], F32, tag="ident")
        make_identity(nc, ident)
        ones = consts.tile([128, 32], F32, tag="ones")
        nc.vector.memset(ones, 1.0)
        eps_t = consts.tile([128, 1], F32, tag="eps")
        nc.vector.memset(eps_t, EPS)
        sexpb = consts.tile([128, 256], F32, tag="sexpb")
        nc.sync.dma_start(sexpb, expb)

        # weights: [128 (d within chunk), 4 (d chunk), 512 (f)]
        sw = {}
        for name, src in (("wq", wq), ("wk", wk), ("wv", wv), ("wo", wo)):
            t = wpool.tile([128, 4, DIM], F32R, tag=name)
            nc.sync.dma_start(t, src.rearrange("(kc p) f -> p kc f", p=128))
            sw[name] = t

        if use_qk_bias:
            sbqk = consts.tile([128, 8], F32, tag="sbqk")
            nc.sync.dma_start(sbqk, bqk)
        if use_v_bias:
            sbv = consts.tile([128, DIM], F32, tag="sbv")
            nc.gpsimd.dma_start(
                sbv, bass.AP(tensor=bv.tensor, offset=0, ap=[[0, 128], [1, DIM]]))
        if use_o_bias:
            sbo = consts.tile([128, DIM], F32, tag="sbo")
            nc.gpsimd.dma_start(
                sbo, bass.AP(tensor=bo.tensor, offset=0, ap=[[0, 128], [1, DIM]]))

        for st in range(ntok // 512):
            t0 = st * 512
            # ---- LayerNorm + transpose: yT [128 d, kc, 512 tok] (f32r)
            yt = ytpool.tile([128, 4, 512], F32R, tag="yt")
            for tt in range(4):
                xt = xpool.tile([128, DIM], F32, tag="xt")
                nc.sync.dma_start(xt, xw[t0 + tt * 128 : t0 + (tt + 1) * 128, :])
                stats = small.tile([128, 6], F32, tag="stats")
                nc.vector.bn_stats(stats, xt)
                mv = small.tile([128, 2], F32, tag="mv")
                nc.vector.bn_aggr(mv, stats)
                # mv[:,1] <- 1/sqrt(var+eps)
                nc.scalar.activation(mv[:, 1:2], mv[:, 1:2],
                                     mybir.ActivationFunctionType.Sqrt,
                                     bias=eps_t, scale=1.0)
                nc.vector.reciprocal(mv[:, 1:2], mv[:, 1:2])
                y = ypool.tile([128, DIM], F32, tag="y")
                nc.vector.tensor_scalar(
                    out=y, in0=xt, scalar1=mv[:, 0:1], scalar2=mv[:, 1:2],
                    op0=mybir.AluOpType.subtract, op1=mybir.AluOpType.mult)
                for kc in range(4):
                    ptr = ps_big.tile([128, 128], F32, tag="big")
                    nc.tensor.transpose(ptr, y[:, kc * 128 : (kc + 1) * 128], ident)
                    nc.vector.tensor_copy(yt[:, kc, tt * 128 : (tt + 1) * 128], ptr)

            # ---- q/k projections -> qkT [128 f, m, 512 tok]  (m: 0-3 q, 4-7 k)
            qkT = qkpool.tile([128, 8, 512], F32, tag="qkT")
            for m in range(8):
                wt = sw["wq"] if m < 4 else sw["wk"]
                mc = m % 4
                pb = ps_big.tile([128, 512], F32, tag="big")
                for kc in range(4):
                    nc.tensor.matmul(pb, wt[:, kc, mc * 128 : (mc + 1) * 128],
                                     yt[:, kc, :], start=(kc == 0), stop=(kc == 3))
                if use_qk_bias:
                    nc.vector.tensor_scalar(
                        out=qkT[:, m, :], in0=pb, scalar1=sbqk[:, m : m + 1],
                        scalar2=None, op0=mybir.AluOpType.add)
                else:
                    nc.vector.tensor_copy(qkT[:, m, :], pb)

            # ---- v projection -> V [128 tok, tt, 512 f]
            V = vpool.tile([128, 4, DIM], F32, tag="V")
            for tt in range(4):
                pb = ps_big.tile([128, 512], F32, tag="big")
                for kc in range(4):
                    nc.tensor.matmul(pb, yt[:, kc, tt * 128 : (tt + 1) * 128],
                                     sw["wv"][:, kc, :],
                                     start=(kc == 0), stop=(kc == 3))
                if use_v_bias:
                    nc.vector.tensor_add(V[:, tt, :], pb, sbv)
                else:
                    nc.vector.tensor_copy(V[:, tt, :], pb)

            # ---- attention, per window-pair tt
            for tt in range(4):
                on = onpool.tile([128, 4, 128], F32R, tag="on")
                for hg in range(4):
                    # sim: simT[j, i] for 4 heads x 2 windows
                    sbanks = []
                    for h in range(4):
                        bank = ps_sim.tile([128, 64], F32, tag="sim")
                        sbanks.append(bank)
                        for wb in range(2):
                            sl = slice(tt * 128 + wb * 64, tt * 128 + wb * 64 + 64)
                            nc.tensor.matmul(
                                bank[wb * 64 : wb * 64 + 64, :],
                                qkT[32 * h : 32 * h + 32, 4 + hg, sl],
                                qkT[32 * h : 32 * h + 32, hg, sl],
                                start=True, stop=True,
                                tile_position=(32 * h, 64 * wb))
                    # P = exp(sim) * exp(bias)
                    P = ppool.tile([128, 256], F32, tag="P")
                    for h in range(4):
                        nc.scalar.activation(P[:, 64 * h : 64 * h + 64], sbanks[h],
                                             mybir.ActivationFunctionType.Exp)
                    nc.gpsimd.tensor_mul(P, P, sexpb)
                    # attn @ V (cols 0:64) and column sums (cols 64:128)
                    for wb in range(2):
                        bank = ps_avs.tile([128, 128], F32, tag="avs")
                        jsl = slice(wb * 64, wb * 64 + 64)
                        for h in range(4):
                            nc.tensor.matmul(
                                bank[32 * h : 32 * h + 32, 0:64],
                                V[wb * 64 : wb * 64 + 64, tt,
                                  hg * 128 + 32 * h : hg * 128 + 32 * h + 32],
                                P[jsl, 64 * h : 64 * h + 64],
                                start=True, stop=True,
                                tile_position=(64 * wb, 32 * h))
                            nc.tensor.matmul(
                                bank[32 * h : 32 * h + 32, 64:128],
                                ones[wb * 64 : wb * 64 + 64, :],
                                P[jsl, 64 * h : 64 * h + 64],
                                start=True, stop=True,
                                tile_position=(64 * wb, 32 * h))
                        rs = rspool.tile([128, 64], F32, tag="rs")
                        nc.vector.reciprocal_approx_fast(out=rs, in_=bank[:, 64:128])
                        nc.vector.tensor_mul(
                            on[:, hg, wb * 64 : wb * 64 + 64], bank[:, 0:64], rs)

                # ---- output projection for this window pair (M=128: both
                # windows share the same rhs, so one accumulation chain)
                fb = ps_big.tile([128, 512], F32, tag="big")
                for hg in range(4):
                    nc.tensor.matmul(fb, on[:, hg, :], sw["wo"][:, hg, :],
                                     start=(hg == 0), stop=(hg == 3))
                fo = fpool.tile([128, DIM], F32, tag="fo")
                if use_o_bias:
                    nc.vector.tensor_add(fo, fb, sbo)
                else:
                    nc.vector.tensor_copy(fo, fb)
                nc.sync.dma_start(out[t0 + tt * 128 : t0 + (tt + 1) * 128, :], fo)

    nc.compile()
    return nc


_PROGRAM_CACHE = {}


def _get_program(flags):
    if flags not in _PROGRAM_CACHE:
        _PROGRAM_CACHE[flags] = _build_program(*flags)
    return _PROGRAM_CACHE[flags]


def _prepare(inputs):
    """Host-side preprocessing. Returns (flags, common_map, per_core_xw)."""
    f32 = lambda a: np.asarray(a, dtype=np.float32)
    x = f32(inputs["x"])
    norm_g, norm_b = f32(inputs["norm_g"]), f32(inputs["norm_b"])
    w_qkv, w_out, b_out = f32(inputs["w_qkv"]), f32(inputs["w_out"]), f32(inputs["b_out"])

    scale = DH ** -0.5
    wq_e = np.ascontiguousarray(norm_g[:, None] * w_qkv[:, 0:DIM] * scale)
    wk_e = np.ascontiguousarray(norm_g[:, None] * w_qkv[:, DIM:2 * DIM])
    wv_e = np.ascontiguousarray(norm_g[:, None] * w_qkv[:, 2 * DIM:3 * DIM])

    bqkv = norm_b @ w_qkv                     # [1536]
    bq = bqkv[0:DIM] * scale
    bk = bqkv[DIM:2 * DIM]
    bv_ = bqkv[2 * DIM:3 * DIM]

    Bt = _dpb_bias_table(
        f32(inputs["dpb_w0"]), f32(inputs["dpb_b0"]), f32(inputs["dpb_ln0"]),
        f32(inputs["dpb_w1"]), f32(inputs["dpb_b1"]), f32(inputs["dpb_ln1"]),
        f32(inputs["dpb_w2"]), f32(inputs["dpb_b2"]), f32(inputs["dpb_ln2"]),
        f32(inputs["dpb_w3"]), f32(inputs["dpb_b3"]))
    # simT[j, i] += B[i, j]  ->  multiplier exp(B.T), tiled [2 windows, 4 heads]
    ebt = np.exp(Bt.T)
    expb = np.ascontiguousarray(
        np.vstack([np.hstack([ebt] * 4)] * 2).astype(np.float32))   # [128, 256]

    use_qk_bias = bool(np.any(bq) or np.any(bk))
    use_v_bias = bool(np.any(bv_))
    use_o_bias = bool(np.any(b_out))
    flags = (use_qk_bias, use_v_bias, use_o_bias)

    common = dict(wq=wq_e, wk=wk_e, wv=wv_e,
                  wo=np.ascontiguousarray(w_out), expb=expb)
    if use_qk_bias:
        common["bqk"] = np.ascontiguousarray(
            np.stack([bq.reshape(4, 128), bk.reshape(4, 128)])
            .reshape(8, 128).T).astype(np.float32)
    if use_v_bias:
        common["bv"] = np.ascontiguousarray(bv_)
    if use_o_bias:
        common["bo"] = np.ascontiguousarray(b_out)

    per_core_xw = [
        _window_partition(x[c * BPC : (c + 1) * BPC]) for c in range(NCORES)
    ]
    return flags, common, per_core_xw


def kernel(**inputs):
    flags, common, per_core_xw = _prepare(inputs)
    nc = _get_program(flags)
    in_maps = [dict(common, xw=per_core_xw[c]) for c in range(NCORES)]
    res = run_bass_kernel_spmd(nc, in_maps, list(range(NCORES)))
    out = np.empty((B, H, W, DIM), np.float32)
    for c in range(NCORES):
        out[c * BPC : (c + 1) * BPC] = _window_unpartition(
            res.results[c]["out"], BPC)
    return out


# revision 17
# speedup vs baseline: 1.4450x; 1.4450x over previous
"""Trainium2 Bass kernel for windowed multi-head attention with dynamic
position bias (nn_Attention_75058848465066).

Contract: kernel(**inputs) takes the FULL unsharded inputs (as produced by
reference.setup_inputs()) and returns the FULL output [16, 64, 64, 512] f32.

Strategy (hardcoded for x [16, 64, 64, 512], 8 cores):
 - Data-parallel over batch: 2 batches -> 128 windows -> 8192 tokens per core.
 - Host: fold layernorm gamma + qk scale into the projection weights, compute
   the tiny DPB MLP -> 64x64 bias table, window-partition x.
 - Device (per core): LN -> PE-transpose -> qkv projections (float32r)
   -> per-window attention with packed (tile_position) small matmuls, softmax
   via exp + ones-matmul column sums (denominator lands pre-broadcast in PSUM)
   -> output projection (float32r) -> windows written back token-major.
"""

import sys

for _p in ("/opt/trn_rl_repo",):
    if _p not in sys.path:
        sys.path.append(_p)

from contextlib import ExitStack

import numpy as np

import concourse.bass as bass
import concourse.tile as tile
from concourse import bacc, mybir
from concourse.bass_utils import run_bass_kernel_spmd
from concourse.masks import make_identity
import concourse.bacc as _bacc_mod

_orig_get_act_tables = _bacc_mod.get_activation_tables


def _single_exp_ln_set(arch):
    tables = _orig_get_act_tables(arch)
    exp = mybir.ActivationFunctionType.Exp
    ln = mybir.ActivationFunctionType.Ln
    if "natural_log_exp_and_others" in tables:
        for name, funcs in tables.items():
            if name != "natural_log_exp_and_others":
                funcs.discard(exp)
                funcs.discard(ln)
    return tables


_bacc_mod.get_activation_tables = _single_exp_ln_set

F32 = mybir.dt.float32
F32R = mybir.dt.float32r
F16 = mybir.dt.float16

DIM = 512
DH = 32
WSZ = 8
HEADS = 16
D4 = DIM // 4
EPS = 1e-5

B, H, W = 16, 64, 64
NCORES = 8
BPC = B // NCORES          # batches per core
NTOK = BPC * H * W         # 8192 tokens per core
WW = WSZ * WSZ             # 64 tokens per window
NST = NTOK // 512          # 16 super-tiles of 512 tokens


# ---------------------------------------------------------------- host math
def _ln_np(x, g, b=None, eps=EPS):
    x = x.astype(np.float32)
    m = x.mean(-1, keepdims=True)
    v = x.var(-1, keepdims=True)
    y = (x - m) * (1.0 / np.sqrt(v + eps)) * g
    return y if b is None else y + b


def _dpb_bias_table(dpb_w0, dpb_b0, dpb_ln0, dpb_w1, dpb_b1, dpb_ln1,
                    dpb_w2, dpb_b2, dpb_ln2, dpb_w3, dpb_b3):
    """Returns B[i, j] = biases[rel_pos_indices[i, j]]  (shape [64, 64])."""
    w = WSZ
    pos = np.arange(w)
    gi, gj = np.meshgrid(pos, pos, indexing="ij")
    grid = np.stack([gi, gj], -1).reshape(-1, 2)
    rel = grid[:, None] - grid[None, :] + (w - 1)
    rel_idx = rel[..., 0] * (2 * w - 1) + rel[..., 1]          # [64, 64]

    p = np.arange(-w, w + 1)
    pi, pj = np.meshgrid(p, p, indexing="ij")
    rp = np.stack([pi, pj], -1).reshape(-1, 2).astype(np.float32)
    h = np.maximum(_ln_np(rp @ dpb_w0 + dpb_b0, dpb_ln0), 0.0)
    h = np.maximum(_ln_np(h @ dpb_w1 + dpb_b1, dpb_ln1), 0.0)
    h = np.maximum(_ln_np(h @ dpb_w2 + dpb_b2, dpb_ln2), 0.0)
    biases = (h @ dpb_w3 + dpb_b3)[:, 0]                       # [(2w-1+2)^2]
    return biases[rel_idx].astype(np.float32)                  # [64, 64]


def _window_partition(x):
    """[b, 64, 64, D] -> [b*64 windows * 64 tokens, D]"""
    b = x.shape[0]
    xw = x.reshape(b, H // WSZ, WSZ, W // WSZ, WSZ, DIM)
    xw = xw.transpose(0, 1, 3, 2, 4, 5)
    return np.ascontiguousarray(xw.reshape(-1, DIM))


def _window_unpartition(xw, b):
    """[b*4096, D] -> [b, 64, 64, D]"""
    x = xw.reshape(b, H // WSZ, W // WSZ, WSZ, WSZ, DIM)
    x = x.transpose(0, 1, 3, 2, 4, 5)
    return np.ascontiguousarray(x.reshape(b, H, W, DIM))


# ------------------------------------------------------------- device build
def _build_program(use_qk_bias, use_v_bias, use_o_bias, ntok=NTOK):
    nc = bacc.Bacc("TRN2", target_bir_lowering=False, debug=False,
                   num_devices=NCORES)

    xw = nc.dram_tensor("xw", [ntok, DIM], F32, kind="ExternalInput").ap()
    wq = nc.dram_tensor("wq", [DIM, DIM], F16, kind="ExternalInput").ap()
    wk = nc.dram_tensor("wk", [DIM, DIM], F16, kind="ExternalInput").ap()
    wv = nc.dram_tensor("wv", [DIM, DIM], F16, kind="ExternalInput").ap()
    wo = nc.dram_tensor("wo", [DIM, DIM], F16, kind="ExternalInput").ap()
    expb = nc.dram_tensor("expb", [128, 256], F32, kind="ExternalInput").ap()
    out = nc.dram_tensor("out", [ntok, DIM], F32, kind="ExternalOutput").ap()
    if use_qk_bias:
        bqk = nc.dram_tensor("bqk", [128, 8], F32, kind="ExternalInput").ap()
    if use_v_bias:
        bv = nc.dram_tensor("bv", [DIM], F32, kind="ExternalInput").ap()
    if use_o_bias:
        bo = nc.dram_tensor("bo", [DIM], F32, kind="ExternalInput").ap()

    with tile.TileContext(nc) as tc, ExitStack() as ctx:
        consts = ctx.enter_context(tc.tile_pool(name="consts", bufs=1))
        wpool = ctx.enter_context(tc.tile_pool(name="wpool", bufs=1))
        xpool = ctx.enter_context(tc.tile_pool(name="xpool", bufs=4))
        ypool = ctx.enter_context(tc.tile_pool(name="ypool", bufs=3))
        small = ctx.enter_context(tc.tile_pool(name="small", bufs=4))
        ytpool = ctx.enter_context(tc.tile_pool(name="ytpool", bufs=3))
        qkpool = ctx.enter_context(tc.tile_pool(name="qkpool", bufs=3))
        vpool = ctx.enter_context(tc.tile_pool(name="vpool", bufs=3))
        ppool = ctx.enter_context(tc.tile_pool(name="ppool", bufs=6))
        rspool = ctx.enter_context(tc.tile_pool(name="rspool", bufs=6))
        onpool = ctx.enter_context(tc.tile_pool(name="onpool", bufs=3))
        fpool = ctx.enter_context(tc.tile_pool(name="fpool", bufs=3))
        ps_big = ctx.enter_context(tc.tile_pool(name="ps_big", bufs=3, space="PSUM"))
        ps_sim = ctx.enter_context(tc.tile_pool(name="ps_sim", bufs=2, space="PSUM"))
        ps_avs = ctx.enter_context(tc.tile_pool(name="ps_avs", bufs=3, space="PSUM"))

        ident = consts.tile([128, 128], F16, tag="ident")
        make_identity(nc, ident)
        ones = consts.tile([128, 32], F16, tag="ones")
        nc.vector.memset(ones, 1.0)
        eps_t = consts.tile([128, 1], F32, tag="eps")
        nc.vector.memset(eps_t, EPS)
        sexpb = consts.tile([128, 256], F16, tag="sexpb")
        nc.gpsimd.dma_start(sexpb, expb)

        # block-diagonal q operand, ping-pong by super-tile parity
        bdq = []
        for par in range(2):
            t = wpool.tile([128, 4, 8, 256], F16, tag=f"bdq{par}")
            nc.vector.memset(t, 0.0)
            bdq.append(t)

        # weights: [128 (d within chunk), 4 (d chunk), 512 (f)]
        sw = {}
        for name, src in (("wq", wq), ("wk", wk), ("wv", wv), ("wo", wo)):
            t = wpool.tile([128, 4, DIM], F16, tag=name)
            nc.sync.dma_start(t, src.rearrange("(kc p) f -> p kc f", p=128))
            sw[name] = t

        if use_qk_bias:
            sbqk = consts.tile([128, 8], F32, tag="sbqk")
            nc.sync.dma_start(sbqk, bqk)
        if use_v_bias:
            sbv = consts.tile([128, DIM], F32, tag="sbv")
            nc.gpsimd.dma_start(
                sbv, bass.AP(tensor=bv.tensor, offset=0, ap=[[0, 128], [1, DIM]]))
        if use_o_bias:
            sbo = consts.tile([128, DIM], F32, tag="sbo")
            nc.gpsimd.dma_start(
                sbo, bass.AP(tensor=bo.tensor, offset=0, ap=[[0, 128], [1, DIM]]))

        for st in range(ntok // 512):
            t0 = st * 512
            # ---- LayerNorm + transpose: yT [128 d, kc, 512 tok] (f32r)
            yt = ytpool.tile([128, 4, 512], F16, tag="yt")
            for tt in range(4):
                xt = xpool.tile([128, DIM], F32, tag="xt")
                nc.sync.dma_start(xt, xw[t0 + tt * 128 : t0 + (tt + 1) * 128, :])
                stats = small.tile([128, 6], F32, tag="stats")
                nc.vector.bn_stats(stats, xt)
                mv = small.tile([128, 2], F32, tag="mv")
                nc.vector.bn_aggr(mv, stats)
                # mv[:,1] <- 1/sqrt(var+eps)
                # rstd = exp(-0.5*ln(var+eps)): keeps ACT on one table set
                nc.scalar.activation(mv[:, 1:2], mv[:, 1:2],
                                     mybir.ActivationFunctionType.Ln,
                                     bias=eps_t, scale=1.0)
                nc.scalar.activation(mv[:, 1:2], mv[:, 1:2],
                                     mybir.ActivationFunctionType.Exp,
                                     bias=0.0, scale=-0.5)
                y = ypool.tile([128, DIM], F16, tag="y")
                nc.vector.tensor_scalar(
                    out=y, in0=xt, scalar1=mv[:, 0:1], scalar2=mv[:, 1:2],
                    op0=mybir.AluOpType.subtract, op1=mybir.AluOpType.mult)
                for kc in range(4):
                    ptr = ps_big.tile([128, 128], F16, tag="big")
                    nc.tensor.transpose(ptr, y[:, kc * 128 : (kc + 1) * 128], ident)
                    nc.vector.tensor_copy(yt[:, kc, tt * 128 : (tt + 1) * 128], ptr)

            # ---- q/k projections -> qkT [128 f, m, 512 tok]  (m: 0-3 q, 4-7 k)
            qkT = qkpool.tile([128, 8, 512], F16, tag="qkT")
            for m in range(8):
                wt = sw["wq"] if m < 4 else sw["wk"]
                mc = m % 4
                pb = ps_big.tile([128, 512], F32, tag="big")
                for kc in range(4):
                    nc.tensor.matmul(pb, wt[:, kc, mc * 128 : (mc + 1) * 128],
                                     yt[:, kc, :], start=(kc == 0), stop=(kc == 3))
                if use_qk_bias:
                    nc.vector.tensor_scalar(
                        out=qkT[:, m, :], in0=pb, scalar1=sbqk[:, m : m + 1],
                        scalar2=None, op0=mybir.AluOpType.add)
                else:
                    nc.scalar.copy(qkT[:, m, :], pb)

            # scatter q head-blocks into the block-diagonal operand
            bq = bdq[st % 2]
            for hg in range(4):
                for h in range(4):
                    nc.sync.dma_start(
                        bq[32 * h : 32 * h + 32, hg, :, 64 * h : 64 * h + 64],
                        qkT[32 * h : 32 * h + 32, hg, :].rearrange(
                            "p (b i) -> p b i", b=8))

            # ---- v projection -> V [128 tok, tt, 512 f]
            V = vpool.tile([128, 4, DIM], F16, tag="V")
            for tt in range(4):
                pb = ps_big.tile([128, 512], F32, tag="big")
                for kc in range(4):
                    nc.tensor.matmul(pb, yt[:, kc, tt * 128 : (tt + 1) * 128],
                                     sw["wv"][:, kc, :],
                                     start=(kc == 0), stop=(kc == 3))
                if use_v_bias:
                    nc.vector.tensor_add(V[:, tt, :], pb, sbv)
                else:
                    nc.scalar.copy(V[:, tt, :], pb)

            # ---- attention, per window-pair tt
            for tt in range(4):
                on = onpool.tile([128, 4, 128], F16, tag="on")
                for hg in range(4):
                    # sim via block-diagonal q: simT[j, (h, i)] in one bank
                    bank = ps_sim.tile([128, 256], F32, tag="sim")
                    for wb in range(2):
                        sl = slice(tt * 128 + wb * 64, tt * 128 + wb * 64 + 64)
                        nc.tensor.matmul(
                            bank[wb * 64 : wb * 64 + 64, :],
                            qkT[:, 4 + hg, sl],
                            bq[:, hg, tt * 2 + wb, :],
                            start=True, stop=True,
                            tile_position=(0, 64 * wb))
                    # P = exp(sim) * exp(bias)
                    P = ppool.tile([128, 256], F16, tag="P")
                    nc.scalar.activation(P, bank,
                                         mybir.ActivationFunctionType.Exp)
                    nc.vector.tensor_mul(P, P, sexpb)
                    # attn @ V (cols 0:64) and column sums (cols 64:128)
                    for wb in range(2):
                        bank = ps_avs.tile([128, 128], F32, tag="avs")
                        jsl = slice(wb * 64, wb * 64 + 64)
                        for h in range(4):
                            nc.tensor.matmul(
                                bank[32 * h : 32 * h + 32, 0:64],
                                V[wb * 64 : wb * 64 + 64, tt,
                                  hg * 128 + 32 * h : hg * 128 + 32 * h + 32],
                                P[jsl, 64 * h : 64 * h + 64],
                                start=True, stop=True,
                                tile_position=(64 * wb, 32 * h))
                            nc.tensor.matmul(
                                bank[32 * h : 32 * h + 32, 64:128],
                                ones[wb * 64 : wb * 64 + 64, :],
                                P[jsl, 64 * h : 64 * h + 64],
                                start=True, stop=True,
                                tile_position=(64 * wb, 32 * h))
                        rs = rspool.tile([128, 64], F32, tag="rs")
                        nc.vector.reciprocal_approx_fast(out=rs, in_=bank[:, 64:128])
                        nc.vector.tensor_mul(
                            on[:, hg, wb * 64 : wb * 64 + 64], bank[:, 0:64], rs)

                # ---- output projection for this window pair (M=128: both
                # windows share the same rhs, so one accumulation chain)
                fb = ps_big.tile([128, 512], F32, tag="big")
                for hg in range(4):
                    nc.tensor.matmul(fb, on[:, hg, :], sw["wo"][:, hg, :],
                                     start=(hg == 0), stop=(hg == 3))
                fo = fpool.tile([128, DIM], F32, tag="fo")
                if use_o_bias:
                    nc.vector.tensor_add(fo, fb, sbo)
                else:
                    nc.scalar.copy(fo, fb)
                nc.sync.dma_start(out[t0 + tt * 128 : t0 + (tt + 1) * 128, :], fo)

    nc.compile()
    return nc


_PROGRAM_CACHE = {}


def _get_program(flags):
    if flags not in _PROGRAM_CACHE:
        _PROGRAM_CACHE[flags] = _build_program(*flags)
    return _PROGRAM_CACHE[flags]


def _prepare(inputs):
    """Host-side preprocessing. Returns (flags, common_map, per_core_xw)."""
    f32 = lambda a: np.asarray(a, dtype=np.float32)
    x = f32(inputs["x"])
    norm_g, norm_b = f32(inputs["norm_g"]), f32(inputs["norm_b"])
    w_qkv, w_out, b_out = f32(inputs["w_qkv"]), f32(inputs["w_out"]), f32(inputs["b_out"])

    scale = DH ** -0.5
    wq_e = np.ascontiguousarray(norm_g[:, None] * w_qkv[:, 0:DIM] * scale)
    wk_e = np.ascontiguousarray(norm_g[:, None] * w_qkv[:, DIM:2 * DIM])
    wv_e = np.ascontiguousarray(norm_g[:, None] * w_qkv[:, 2 * DIM:3 * DIM])

    bqkv = norm_b @ w_qkv                     # [1536]
    bq = bqkv[0:DIM] * scale
    bk = bqkv[DIM:2 * DIM]
    bv_ = bqkv[2 * DIM:3 * DIM]

    Bt = _dpb_bias_table(
        f32(inputs["dpb_w0"]), f32(inputs["dpb_b0"]), f32(inputs["dpb_ln0"]),
        f32(inputs["dpb_w1"]), f32(inputs["dpb_b1"]), f32(inputs["dpb_ln1"]),
        f32(inputs["dpb_w2"]), f32(inputs["dpb_b2"]), f32(inputs["dpb_ln2"]),
        f32(inputs["dpb_w3"]), f32(inputs["dpb_b3"]))
    # simT[j, i] += B[i, j]  ->  multiplier exp(B.T), tiled [2 windows, 4 heads]
    ebt = np.exp(Bt.T)
    expb = np.ascontiguousarray(
        np.vstack([np.hstack([ebt] * 4)] * 2).astype(np.float32))   # [128, 256]

    use_qk_bias = bool(np.any(bq) or np.any(bk))
    use_v_bias = bool(np.any(bv_))
    use_o_bias = bool(np.any(b_out))
    flags = (use_qk_bias, use_v_bias, use_o_bias)

    f16 = lambda a: np.ascontiguousarray(a).astype(np.float16)
    common = dict(wq=f16(wq_e), wk=f16(wk_e), wv=f16(wv_e),
                  wo=f16(w_out), expb=expb)
    if use_qk_bias:
        common["bqk"] = np.ascontiguousarray(
            np.stack([bq.reshape(4, 128), bk.reshape(4, 128)])
            .reshape(8, 128).T).astype(np.float32)
    if use_v_bias:
        common["bv"] = np.ascontiguousarray(bv_)
    if use_o_bias:
        common["bo"] = np.ascontiguousarray(b_out)

    per_core_xw = [
        _window_partition(x[c * BPC : (c + 1) * BPC]) for c in range(NCORES)
    ]
    return flags, common, per_core_xw


def kernel(**inputs):
    flags, common, per_core_xw = _prepare(inputs)
    nc = _get_program(flags)
    in_maps = [dict(common, xw=per_core_xw[c]) for c in range(NCORES)]
    res = run_bass_kernel_spmd(nc, in_maps, list(range(NCORES)))
    out = np.empty((B, H, W, DIM), np.float32)
    for c in range(NCORES):
        out[c * BPC : (c + 1) * BPC] = _window_unpartition(
            res.results[c]["out"], BPC)
    return out


# revision 18
# speedup vs baseline: 1.7351x; 1.2008x over previous
"""Trainium2 Bass kernel for windowed multi-head attention with dynamic
position bias (nn_Attention_75058848465066).

Contract: kernel(**inputs) takes the FULL unsharded inputs (as produced by
reference.setup_inputs()) and returns the FULL output [16, 64, 64, 512] f32.

Strategy (hardcoded for x [16, 64, 64, 512], 8 cores):
 - Data-parallel over batch: 2 batches -> 128 windows -> 8192 tokens per core.
 - Host: fold layernorm gamma + qk scale into the projection weights, compute
   the tiny DPB MLP -> 64x64 bias table, window-partition x.
 - Device (per core): LN -> PE-transpose -> qkv projections (float32r)
   -> per-window attention with packed (tile_position) small matmuls, softmax
   via exp + ones-matmul column sums (denominator lands pre-broadcast in PSUM)
   -> output projection (float32r) -> windows written back token-major.
"""

import sys

for _p in ("/opt/trn_rl_repo",):
    if _p not in sys.path:
        sys.path.append(_p)

from contextlib import ExitStack

import numpy as np

import concourse.bass as bass
import concourse.tile as tile
from concourse import bacc, mybir
from concourse.bass_utils import run_bass_kernel_spmd
from concourse.masks import make_identity
import concourse.bacc as _bacc_mod

_orig_get_act_tables = _bacc_mod.get_activation_tables


def _single_exp_ln_set(arch):
    tables = _orig_get_act_tables(arch)
    exp = mybir.ActivationFunctionType.Exp
    ln = mybir.ActivationFunctionType.Ln
    if "natural_log_exp_and_others" in tables:
        for name, funcs in tables.items():
            if name != "natural_log_exp_and_others":
                funcs.discard(exp)
                funcs.discard(ln)
    return tables


_bacc_mod.get_activation_tables = _single_exp_ln_set

F32 = mybir.dt.float32
F32R = mybir.dt.float32r
F16 = mybir.dt.float16

DIM = 512
DH = 32
WSZ = 8
HEADS = 16
D4 = DIM // 4
EPS = 1e-5

B, H, W = 16, 64, 64
NCORES = 8
BPC = B // NCORES          # batches per core
NTOK = BPC * H * W         # 8192 tokens per core
WW = WSZ * WSZ             # 64 tokens per window
NST = NTOK // 512          # 16 super-tiles of 512 tokens


# ---------------------------------------------------------------- host math
def _ln_np(x, g, b=None, eps=EPS):
    x = x.astype(np.float32)
    m = x.mean(-1, keepdims=True)
    v = x.var(-1, keepdims=True)
    y = (x - m) * (1.0 / np.sqrt(v + eps)) * g
    return y if b is None else y + b


def _dpb_bias_table(dpb_w0, dpb_b0, dpb_ln0, dpb_w1, dpb_b1, dpb_ln1,
                    dpb_w2, dpb_b2, dpb_ln2, dpb_w3, dpb_b3):
    """Returns B[i, j] = biases[rel_pos_indices[i, j]]  (shape [64, 64])."""
    w = WSZ
    pos = np.arange(w)
    gi, gj = np.meshgrid(pos, pos, indexing="ij")
    grid = np.stack([gi, gj], -1).reshape(-1, 2)
    rel = grid[:, None] - grid[None, :] + (w - 1)
    rel_idx = rel[..., 0] * (2 * w - 1) + rel[..., 1]          # [64, 64]

    p = np.arange(-w, w + 1)
    pi, pj = np.meshgrid(p, p, indexing="ij")
    rp = np.stack([pi, pj], -1).reshape(-1, 2).astype(np.float32)
    h = np.maximum(_ln_np(rp @ dpb_w0 + dpb_b0, dpb_ln0), 0.0)
    h = np.maximum(_ln_np(h @ dpb_w1 + dpb_b1, dpb_ln1), 0.0)
    h = np.maximum(_ln_np(h @ dpb_w2 + dpb_b2, dpb_ln2), 0.0)
    biases = (h @ dpb_w3 + dpb_b3)[:, 0]                       # [(2w-1+2)^2]
    return biases[rel_idx].astype(np.float32)                  # [64, 64]


def _window_partition(x):
    """[b, 64, 64, D] -> [b*64 windows * 64 tokens, D]"""
    b = x.shape[0]
    xw = x.reshape(b, H // WSZ, WSZ, W // WSZ, WSZ, DIM)
    xw = xw.transpose(0, 1, 3, 2, 4, 5)
    return np.ascontiguousarray(xw.reshape(-1, DIM))


def _window_unpartition(xw, b):
    """[b*4096, D] -> [b, 64, 64, D]"""
    x = xw.reshape(b, H // WSZ, W // WSZ, WSZ, WSZ, DIM)
    x = x.transpose(0, 1, 3, 2, 4, 5)
    return np.ascontiguousarray(x.reshape(b, H, W, DIM))


# ------------------------------------------------------------- device build
def _build_program(use_qk_bias, use_v_bias, use_o_bias, ntok=NTOK):
    nc = bacc.Bacc("TRN2", target_bir_lowering=False, debug=False,
                   num_devices=NCORES)

    xw = nc.dram_tensor("xw", [ntok, DIM], F32, kind="ExternalInput").ap()
    wq = nc.dram_tensor("wq", [DIM, DIM], F16, kind="ExternalInput").ap()
    wk = nc.dram_tensor("wk", [DIM, DIM], F16, kind="ExternalInput").ap()
    wv = nc.dram_tensor("wv", [DIM, DIM], F16, kind="ExternalInput").ap()
    wo = nc.dram_tensor("wo", [DIM, DIM], F16, kind="ExternalInput").ap()
    expb = nc.dram_tensor("expb", [128, 256], F32, kind="ExternalInput").ap()
    out = nc.dram_tensor("out", [ntok, DIM], F32, kind="ExternalOutput").ap()
    if use_qk_bias:
        bqk = nc.dram_tensor("bqk", [128, 8], F32, kind="ExternalInput").ap()
    if use_v_bias:
        bv = nc.dram_tensor("bv", [DIM], F32, kind="ExternalInput").ap()
    if use_o_bias:
        bo = nc.dram_tensor("bo", [DIM], F32, kind="ExternalInput").ap()

    with tile.TileContext(nc) as tc, ExitStack() as ctx:
        consts = ctx.enter_context(tc.tile_pool(name="consts", bufs=1))
        wpool = ctx.enter_context(tc.tile_pool(name="wpool", bufs=1))
        xpool = ctx.enter_context(tc.tile_pool(name="xpool", bufs=4))
        ypool = ctx.enter_context(tc.tile_pool(name="ypool", bufs=3))
        small = ctx.enter_context(tc.tile_pool(name="small", bufs=4))
        ytpool = ctx.enter_context(tc.tile_pool(name="ytpool", bufs=3))
        qkpool = ctx.enter_context(tc.tile_pool(name="qkpool", bufs=3))
        vpool = ctx.enter_context(tc.tile_pool(name="vpool", bufs=3))
        ppool = ctx.enter_context(tc.tile_pool(name="ppool", bufs=6))
        rspool = ctx.enter_context(tc.tile_pool(name="rspool", bufs=6))
        onpool = ctx.enter_context(tc.tile_pool(name="onpool", bufs=3))
        fpool = ctx.enter_context(tc.tile_pool(name="fpool", bufs=3))
        ps_big = ctx.enter_context(tc.tile_pool(name="ps_big", bufs=3, space="PSUM"))
        ps_sim = ctx.enter_context(tc.tile_pool(name="ps_sim", bufs=2, space="PSUM"))
        ps_avs = ctx.enter_context(tc.tile_pool(name="ps_avs", bufs=3, space="PSUM"))

        ident = consts.tile([128, 128], F16, tag="ident")
        make_identity(nc, ident)
        ones = consts.tile([128, 32], F16, tag="ones")
        nc.vector.memset(ones, 1.0)
        eps_t = consts.tile([128, 1], F32, tag="eps")
        nc.vector.memset(eps_t, EPS)
        sexpb = consts.tile([128, 256], F16, tag="sexpb")
        nc.gpsimd.dma_start(sexpb, expb)

        # block-diagonal q operand, ping-pong by super-tile parity
        bdq = []
        for par in range(2):
            t = wpool.tile([128, 4, 8, 256], F16, tag=f"bdq{par}")
            nc.vector.memset(t, 0.0)
            bdq.append(t)

        # weights: [128 (d within chunk), 4 (d chunk), 512 (f)]
        sw = {}
        for name, src in (("wq", wq), ("wk", wk), ("wv", wv), ("wo", wo)):
            t = wpool.tile([128, 4, DIM], F16, tag=name)
            nc.sync.dma_start(t, src.rearrange("(kc p) f -> p kc f", p=128))
            sw[name] = t

        if use_qk_bias:
            sbqk = consts.tile([128, 8], F32, tag="sbqk")
            nc.sync.dma_start(sbqk, bqk)
        if use_v_bias:
            sbv = consts.tile([128, DIM], F32, tag="sbv")
            nc.gpsimd.dma_start(
                sbv, bass.AP(tensor=bv.tensor, offset=0, ap=[[0, 128], [1, DIM]]))
        if use_o_bias:
            sbo = consts.tile([128, DIM], F32, tag="sbo")
            nc.gpsimd.dma_start(
                sbo, bass.AP(tensor=bo.tensor, offset=0, ap=[[0, 128], [1, DIM]]))

        for st in range(ntok // 512):
            t0 = st * 512
            # ---- LayerNorm + transpose: yT [128 d, kc, 512 tok] (f32r)
            yt = ytpool.tile([128, 4, 512], F16, tag="yt")
            for tt in range(4):
                xt = xpool.tile([128, DIM], F32, tag="xt")
                nc.sync.dma_start(xt, xw[t0 + tt * 128 : t0 + (tt + 1) * 128, :])
                stats = small.tile([128, 6], F32, tag="stats")
                nc.vector.bn_stats(stats, xt)
                mv = small.tile([128, 2], F32, tag="mv")
                nc.vector.bn_aggr(mv, stats)
                # mv[:,1] <- 1/sqrt(var+eps)
                # rstd = exp(-0.5*ln(var+eps)): keeps ACT on one table set
                nc.scalar.activation(mv[:, 1:2], mv[:, 1:2],
                                     mybir.ActivationFunctionType.Ln,
                                     bias=eps_t, scale=1.0)
                nc.scalar.activation(mv[:, 1:2], mv[:, 1:2],
                                     mybir.ActivationFunctionType.Exp,
                                     bias=0.0, scale=-0.5)
                y = ypool.tile([128, DIM], F16, tag="y")
                nc.vector.tensor_scalar(
                    out=y, in0=xt, scalar1=mv[:, 0:1], scalar2=mv[:, 1:2],
                    op0=mybir.AluOpType.subtract, op1=mybir.AluOpType.mult)
                for kc in range(4):
                    ptr = ps_big.tile([128, 128], F16, tag="big")
                    nc.tensor.transpose(ptr, y[:, kc * 128 : (kc + 1) * 128], ident)
                    nc.vector.tensor_copy(yt[:, kc, tt * 128 : (tt + 1) * 128], ptr)

            # ---- q/k projections -> qkT [128 f, m, 512 tok]  (m: 0-3 q, 4-7 k)
            qkT = qkpool.tile([128, 8, 512], F16, tag="qkT")
            bq = bdq[st % 2]
            for m in range(8):
                wt = sw["wq"] if m < 4 else sw["wk"]
                mc = m % 4
                pb = ps_big.tile([128, 512], F32, tag="big")
                for kc in range(4):
                    nc.tensor.matmul(pb, wt[:, kc, mc * 128 : (mc + 1) * 128],
                                     yt[:, kc, :], start=(kc == 0), stop=(kc == 3))
                if use_qk_bias:
                    nc.vector.tensor_scalar(
                        out=qkT[:, m, :], in0=pb, scalar1=sbqk[:, m : m + 1],
                        scalar2=None, op0=mybir.AluOpType.add)
                elif m % 2 == 0:
                    nc.scalar.copy(qkT[:, m, :], pb)
                else:
                    nc.vector.tensor_copy(qkT[:, m, :], pb)
                if m < 4:
                    # scatter this q chunk into its block-diagonal operand now
                    for h in range(4):
                        nc.sync.dma_start(
                            bq[32 * h : 32 * h + 32, m, :, 64 * h : 64 * h + 64],
                            qkT[32 * h : 32 * h + 32, m, :].rearrange(
                                "p (b i) -> p b i", b=8))

            # ---- v projection -> V [128 tok, tt, 512 f]
            V = vpool.tile([128, 4, DIM], F16, tag="V")
            for tt in range(4):
                pb = ps_big.tile([128, 512], F32, tag="big")
                for kc in range(4):
                    nc.tensor.matmul(pb, yt[:, kc, tt * 128 : (tt + 1) * 128],
                                     sw["wv"][:, kc, :],
                                     start=(kc == 0), stop=(kc == 3))
                if use_v_bias:
                    nc.vector.tensor_add(V[:, tt, :], pb, sbv)
                else:
                    nc.scalar.copy(V[:, tt, :], pb)

            # ---- attention, per window-pair tt
            for tt in range(4):
                on = onpool.tile([128, 4, 128], F16, tag="on")
                for hg in range(4):
                    # sim via block-diagonal q: simT[j, (h, i)] in one bank
                    bank = ps_sim.tile([128, 256], F32, tag="sim")
                    for wb in range(2):
                        sl = slice(tt * 128 + wb * 64, tt * 128 + wb * 64 + 64)
                        nc.tensor.matmul(
                            bank[wb * 64 : wb * 64 + 64, :],
                            qkT[:, 4 + hg, sl],
                            bq[:, hg, tt * 2 + wb, :],
                            start=True, stop=True,
                            tile_position=(0, 64 * wb))
                    # P = exp(sim) * exp(bias)
                    P = ppool.tile([128, 256], F16, tag="P")
                    nc.scalar.activation(P, bank,
                                         mybir.ActivationFunctionType.Exp)
                    nc.vector.tensor_mul(P, P, sexpb)
                    # attn @ V (cols 0:64) and column sums (cols 64:128)
                    for wb in range(2):
                        bank = ps_avs.tile([128, 128], F32, tag="avs")
                        jsl = slice(wb * 64, wb * 64 + 64)
                        for h in range(4):
                            nc.tensor.matmul(
                                bank[32 * h : 32 * h + 32, 0:64],
                                V[wb * 64 : wb * 64 + 64, tt,
                                  hg * 128 + 32 * h : hg * 128 + 32 * h + 32],
                                P[jsl, 64 * h : 64 * h + 64],
                                start=True, stop=True,
                                tile_position=(64 * wb, 32 * h))
                            nc.tensor.matmul(
                                bank[32 * h : 32 * h + 32, 64:128],
                                ones[wb * 64 : wb * 64 + 64, :],
                                P[jsl, 64 * h : 64 * h + 64],
                                start=True, stop=True,
                                tile_position=(64 * wb, 32 * h))
                        rs = rspool.tile([128, 64], F32, tag="rs")
                        nc.vector.reciprocal_approx_fast(out=rs, in_=bank[:, 64:128])
                        nc.vector.tensor_mul(
                            on[:, hg, wb * 64 : wb * 64 + 64], bank[:, 0:64], rs)

                # ---- output projection for this window pair (M=128: both
                # windows share the same rhs, so one accumulation chain)
                fb = ps_big.tile([128, 512], F32, tag="big")
                for hg in range(4):
                    nc.tensor.matmul(fb, on[:, hg, :], sw["wo"][:, hg, :],
                                     start=(hg == 0), stop=(hg == 3))
                fo = fpool.tile([128, DIM], F32, tag="fo")
                if use_o_bias:
                    nc.vector.tensor_add(fo, fb, sbo)
                else:
                    nc.scalar.copy(fo, fb)
                nc.sync.dma_start(out[t0 + tt * 128 : t0 + (tt + 1) * 128, :], fo)

    nc.compile()
    return nc


_PROGRAM_CACHE = {}


def _get_program(flags):
    if flags not in _PROGRAM_CACHE:
        _PROGRAM_CACHE[flags] = _build_program(*flags)
    return _PROGRAM_CACHE[flags]


def _prepare(inputs):
    """Host-side preprocessing. Returns (flags, common_map, per_core_xw)."""
    f32 = lambda a: np.asarray(a, dtype=np.float32)
    x = f32(inputs["x"])
    norm_g, norm_b = f32(inputs["norm_g"]), f32(inputs["norm_b"])
    w_qkv, w_out, b_out = f32(inputs["w_qkv"]), f32(inputs["w_out"]), f32(inputs["b_out"])

    scale = DH ** -0.5
    wq_e = np.ascontiguousarray(norm_g[:, None] * w_qkv[:, 0:DIM] * scale)
    wk_e = np.ascontiguousarray(norm_g[:, None] * w_qkv[:, DIM:2 * DIM])
    wv_e = np.ascontiguousarray(norm_g[:, None] * w_qkv[:, 2 * DIM:3 * DIM])

    bqkv = norm_b @ w_qkv                     # [1536]
    bq = bqkv[0:DIM] * scale
    bk = bqkv[DIM:2 * DIM]
    bv_ = bqkv[2 * DIM:3 * DIM]

    Bt = _dpb_bias_table(
        f32(inputs["dpb_w0"]), f32(inputs["dpb_b0"]), f32(inputs["dpb_ln0"]),
        f32(inputs["dpb_w1"]), f32(inputs["dpb_b1"]), f32(inputs["dpb_ln1"]),
        f32(inputs["dpb_w2"]), f32(inputs["dpb_b2"]), f32(inputs["dpb_ln2"]),
        f32(inputs["dpb_w3"]), f32(inputs["dpb_b3"]))
    # simT[j, i] += B[i, j]  ->  multiplier exp(B.T), tiled [2 windows, 4 heads]
    ebt = np.exp(Bt.T)
    expb = np.ascontiguousarray(
        np.vstack([np.hstack([ebt] * 4)] * 2).astype(np.float32))   # [128, 256]

    use_qk_bias = bool(np.any(bq) or np.any(bk))
    use_v_bias = bool(np.any(bv_))
    use_o_bias = bool(np.any(b_out))
    flags = (use_qk_bias, use_v_bias, use_o_bias)

    f16 = lambda a: np.ascontiguousarray(a).astype(np.float16)
    common = dict(wq=f16(wq_e), wk=f16(wk_e), wv=f16(wv_e),
                  wo=f16(w_out), expb=expb)
    if use_qk_bias:
        common["bqk"] = np.ascontiguousarray(
            np.stack([bq.reshape(4, 128), bk.reshape(4, 128)])
            .reshape(8, 128).T).astype(np.float32)
    if use_v_bias:
        common["bv"] = np.ascontiguousarray(bv_)
    if use_o_bias:
        common["bo"] = np.ascontiguousarray(b_out)

    per_core_xw = [
        _window_partition(x[c * BPC : (c + 1) * BPC]) for c in range(NCORES)
    ]
    return flags, common, per_core_xw


def kernel(**inputs):
    flags, common, per_core_xw = _prepare(inputs)
    nc = _get_program(flags)
    in_maps = [dict(common, xw=per_core_xw[c]) for c in range(NCORES)]
    res = run_bass_kernel_spmd(nc, in_maps, list(range(NCORES)))
    out = np.empty((B, H, W, DIM), np.float32)
    for c in range(NCORES):
        out[c * BPC : (c + 1) * BPC] = _window_unpartition(
            res.results[c]["out"], BPC)
    return out
